# revision 40
# baseline (speedup 1.0000x reference)
"""Trainium2 Bass kernel for the trajectory-decoder LSTM problem.

Math (mirrors the reference, with algebraic folds):
  dec_inp feeds gates only through W_emb; W_sp/W_emb/W_hp collapse:
    W_es = W_emb @ W_sp            [4H, 2]
    gates_t = zx@W_zx.T + bias + r_{t-1}@W_es.T + h_{t-1}@W_hh.T
  For t>=1, r_{t-1} = h_{t-1}@W_hp.T + b_hp, so with
    W_hh' = W_hh + W_es @ W_hp,  bias1 = b_ih + b_hh + W_emb@b_sp + W_es@b_hp
  every step becomes uniform:  gates_t = zx@W_zx.T + bias1 + h_{t-1}@W_hh'.T
  plus a rank-2 step-0 correction (lpr - r_init)@W_es.T injected once.
  `last_pos` is dead code (output is just the stacked rel_pos).

  Cell-state folding: track ct = c/2.  With m1 = (sig(2g)-0.5)*sig(i)
  (= sig(i)*tanh(g)/2) and m2 = sig(f)*ct_prev:
    ct = m1 + m2,   tanh(c) = tanh(2*ct)  (ACT scale=2)
  so the c-update is a plain tensor_tensor add and no x2 op is needed.

  bias1 and b1 ride a constant-1 feature appended to zx (row ZX=1056 of
  the zxT layout): W_zx.T row 1056 = bias1, W1.T row 1056 = b1.  No
  bank-open bias matmuls, no bias adds on the vector engines.

Device strategy (pure data-parallel over 8 cores, 4096 batch each):
  - batch on the free dim, features on partitions
  - per 256-batch wave, the 4 gate pre-activations live RESIDENT in one
    2-bank PSUM tile; each step the PE accumulates (h_t - h_{t-1}) @ W_hh'.T
    into it (start=False).  First zx matmul per gate block opens the bank
    (start=True).
  - g-gate weights doubled on host so ONE sigmoid over all 4 blocks
    yields sig(i),sig(f),sig(o),sig(2g).
  - micro-round software pipeline: one chain-step per micro-round, three
    chains in flight (stagger 12 micros, step period 3 micros).  Engine
    assignment: ACT {sigmoid, tanh, half the rel copies}, DVE {m1, ct, h,
    mlp relu, other rel copies}, GPS {m2, dh}.  Emission order per engine
    is staggered so ops are ready when the engine reaches them.
  - all matmul operands fp16, PSUM accumulation fp32; b_hp added on host.
"""

import os
import numpy as np

B = 32768
NCORES = 8
BC = B // NCORES          # 4096 batch per core
WAVE = 256                # batch per recurrence chain (2 PSUM banks of gates)
NW = BC // WAVE           # 16 waves
PAIR = 2 * WAVE           # phase-A (mlp) runs at N=512 across wave pairs
T = 12                    # decode steps
H = 128
G4 = 4 * H                # 512 gate features
ZX = 1056
KP = 1152                 # ZX+1 (bias row) padded to 9*128
KT = KP // 128            # 9 contraction tiles
MLP = 1024
EMB = 64

# Stagger between chain starts: the step period is 3 micros, so the three
# in-flight chains must land on distinct micro residues mod 3, while
# S(w+3)-S(w) = 36 = chain span lets chain w+3 inherit chain w's rotation
# slot (and PSUM gates buffer) with no idle micros.  [13,13,10] does both.
STAG_PAT = [13, 13, 10]
LEAD = 11                 # micros of mlp warm-up before chain 0

_cache = {}


def _build_nc():
    import concourse.bass as bass
    import concourse.bacc as bacc
    import concourse.mybir as mybir
    import concourse.tile as tile
    from concourse.bass import ts

    f16 = mybir.dt.float16
    f32 = mybir.dt.float32
    AF = mybir.ActivationFunctionType
    OP = mybir.AluOpType

    nc = bacc.Bacc("TRN2", target_bir_lowering=False)

    zxT = nc.dram_tensor("zxT", [KP, BC], f16, kind="ExternalInput")
    lprT = nc.dram_tensor("lprT", [2, BC], f16, kind="ExternalInput")
    w1t = nc.dram_tensor("w1t", [128, KT, MLP], f16, kind="ExternalInput")
    wzxt = nc.dram_tensor("wzxt", [128, KT, G4], f16, kind="ExternalInput")
    w2t = nc.dram_tensor("w2t", [128, 8, H], f16, kind="ExternalInput")
    whht = nc.dram_tensor("whht", [128, G4], f16, kind="ExternalInput")
    whpt = nc.dram_tensor("whpt", [128, 2], f16, kind="ExternalInput")
    k3 = nc.dram_tensor("k3", [2, G4], f16, kind="ExternalInput")   # -W_es.T
    wes = nc.dram_tensor("wes", [2, G4], f16, kind="ExternalInput")  # +W_es.T
    bias2 = nc.dram_tensor("bias2", [2, 2 * 128], f16, kind="ExternalInput")
    ind = nc.dram_tensor("ind", [2, 2 * WAVE], f16, kind="ExternalInput")
    b2 = nc.dram_tensor("b2", [128, 1], f32, kind="ExternalInput")
    bhp = nc.dram_tensor("bhp", [2, 1], f32, kind="ExternalInput")
    pred = nc.dram_tensor("pred", [T, 2, BC], f32, kind="ExternalOutput")

    with tile.TileContext(nc) as tc:
        with (
            tc.tile_pool(name="consts", bufs=1) as cpool,
            tc.tile_pool(name="zx", bufs=2) as zxpool,
            tc.tile_pool(name="h1", bufs=2) as h1pool,
            tc.tile_pool(name="hc", bufs=10) as hcpool,
            tc.tile_pool(name="acts", bufs=6) as apool,
            tc.tile_pool(name="outs", bufs=4) as opool,
            tc.tile_pool(name="scrps", bufs=2, space="PSUM") as scrpool,
            tc.tile_pool(name="gateps", bufs=3, space="PSUM") as gatepool,
        ):
            # ---- load constants once (small ones first: the warmup matmuls
            # and first mlp chunks shouldn't wait behind megabyte weights) ----
            ind_s = cpool.tile([2, 2 * WAVE], f16)
            nc.sync.dma_start(ind_s[:], ind[:])
            bias2_s = cpool.tile([2, 2 * 128], f16)
            nc.sync.dma_start(bias2_s[:], bias2[:])
            whht_s = cpool.tile([128, G4], f16)
            nc.sync.dma_start(whht_s[:], whht[:])
            whpt_s = cpool.tile([128, 2], f16)
            nc.sync.dma_start(whpt_s[:], whpt[:])
            k3_s = cpool.tile([2, G4], f16)
            nc.sync.dma_start(k3_s[:], k3[:])
            wes_s = cpool.tile([2, G4], f16)
            nc.sync.dma_start(wes_s[:], wes[:])
            b2_s = cpool.tile([128, 1], f32)
            nc.sync.dma_start(b2_s[:], b2[:])
            bhp_s = cpool.tile([2, 1], f32)
            nc.sync.dma_start(bhp_s[:], bhp[:])
            lpr_s = cpool.tile([2, BC], f16)
            nc.sync.dma_start(lpr_s[:], lprT[:])
            w1t_s = cpool.tile([128, KT, MLP], f16)
            for j in range(8):
                nc.sync.dma_start(
                    w1t_s[:, :, ts(j, 128)], w1t[:, :, ts(j, 128)]
                )
            w2t_s = cpool.tile([128, 8, H], f16)
            nc.sync.dma_start(w2t_s[:], w2t[:])
            wzxt_s = cpool.tile([128, KT, G4], f16)
            nc.sync.dma_start(wzxt_s[:, :, 0 : 2 * 128], wzxt[:, :, 0 : 2 * 128])
            nc.sync.dma_start(
                wzxt_s[:, :, 2 * 128 : G4], wzxt[:, :, 2 * 128 : G4]
            )

            zxT_v = zxT.rearrange("(k p) b -> p k b", p=128)
            pred_v = pred.rearrange("t j b -> j t b")

            state = [dict() for _ in range(NW)]
            pair_state = [dict() for _ in range(NW // 2)]
            events = []  # (micro, prio, fn)

            def mk_mlp1_c(p, j, klo, khi):
                def fn():
                    st = pair_state[p]
                    if "zxw" not in st:
                        zxw = zxpool.tile([128, KT, PAIR], f16, tag="zxw", name="zxw")
                        nc.sync.dma_start(zxw[:], zxT_v[:, :, ts(p, PAIR)])
                        st["zxw"] = zxw
                        st["h1"] = h1pool.tile([128, 8, PAIR], f16, tag="h1", name="h1")
                    if klo == 0:
                        st["ps%d" % j] = scrpool.tile(
                            [128, PAIR], f32, tag="scratch", name="ps"
                        )
                    ps = st["ps%d" % j]
                    for k in range(klo, khi):
                        nc.tensor.matmul(
                            ps[:], w1t_s[:, k, ts(j, 128)], st["zxw"][:, k, :],
                            start=(k == 0), stop=(k == KT - 1),
                        )
                return fn

            def mk_relu1(p, j):
                def fn():
                    st = pair_state[p]
                    # b1 rides the constant-1 zx row: pure relu here
                    nc.vector.tensor_scalar_max(
                        st["h1"][:, j, :], st.pop("ps%d" % j)[:], 0.0
                    )
                return fn

            def mk_mlp2_c(p, jlo, jhi):
                def fn():
                    st = pair_state[p]
                    if jlo == 0:
                        st["ps2"] = scrpool.tile(
                            [128, PAIR], f32, tag="scratch", name="ps"
                        )
                    ps = st["ps2"]
                    for j in range(jlo, jhi):
                        nc.tensor.matmul(
                            ps[:], w2t_s[:, j, :], st["h1"][:, j, :],
                            start=(j == 0), stop=(j == 7),
                        )
                return fn

            def mk_relu2(p):
                def fn():
                    st = pair_state[p]
                    hi = h1pool.tile([128, PAIR], f16, tag="hinit", name="hinit")
                    nc.vector.tensor_scalar(
                        hi[:], st.pop("ps2")[:], b2_s[:, 0:1], 0.0, OP.add, OP.max
                    )
                    st["h_init"] = hi
                return fn

            def mk_init1(w):
                def fn():
                    st = state[w]
                    pst = pair_state[w // 2]
                    hs = ts(w % 2, WAVE)
                    st["h_prev"] = pst["h_init"][:, hs]
                    gates = gatepool.tile([128, 4 * WAVE], f32, tag="gates", name="gates")
                    st["gates"] = gates
                    # bank-open: start=True must cover a whole PSUM bank, so
                    # bias1 is injected via a K=2 matmul against a 0/1 mask
                    for bk in range(2):
                        nc.tensor.matmul(
                            gates[:, ts(bk, 2 * WAVE)], bias2_s[:, ts(bk, 128)],
                            ind_s[:], start=True, stop=False, skip_group_check=True,
                        )
                    for g in range(2):
                        gp = gates[:, ts(g, WAVE)]
                        for k in range(KT):
                            nc.tensor.matmul(
                                gp[:], wzxt_s[:, k, ts(g, 128)],
                                pst["zxw"][:, k, hs],
                                start=False, stop=False, skip_group_check=True,
                            )
                return fn

            def mk_init2(w):
                def fn():
                    st = state[w]
                    pst = pair_state[w // 2]
                    hs = ts(w % 2, WAVE)
                    gates = st["gates"]
                    for g in range(2, 4):
                        gp = gates[:, ts(g, WAVE)]
                        for k in range(KT):
                            nc.tensor.matmul(
                                gp[:], wzxt_s[:, k, ts(g, 128)],
                                pst["zxw"][:, k, hs],
                                start=False, stop=False, skip_group_check=True,
                            )
                    psr = scrpool.tile([2, WAVE], f32, tag="scratch", name="psr")
                    nc.tensor.matmul(
                        psr[:], whpt_s[:], st["h_prev"][:], start=True, stop=True
                    )
                    st["psr"] = psr
                    for g in range(4):
                        nc.tensor.matmul(
                            gates[:, ts(g, WAVE)], whht_s[:, ts(g, 128)],
                            st["h_prev"][:],
                            start=False, stop=False, skip_group_check=True,
                        )
                    st["predsb"] = opool.tile([2, T * WAVE], f32, tag="predsb", name="predsb")
                    st["h_all"] = opool.tile([128, T * WAVE], f16, tag="hall", name="hall")
                return fn

            def mk_k3rhs(w):
                def fn():
                    st = state[w]
                    k3rhs = apool.tile([2, WAVE], f16, tag="k3rhs", name="k3rhs")
                    nc.vector.scalar_tensor_tensor(
                        k3rhs[:], st.pop("psr")[:], bhp_s[:, 0:1],
                        lpr_s[:, ts(w, WAVE)], OP.add, OP.subtract,
                    )
                    st["k3rhs"] = k3rhs
                return fn

            def mk_k3mm(w):
                def fn():
                    st = state[w]
                    gates = st["gates"]
                    for g in range(4):
                        nc.tensor.matmul(
                            gates[:, ts(g, WAVE)], k3_s[:, ts(g, 128)],
                            st["k3rhs"][:],
                            start=False, stop=False, skip_group_check=True,
                        )
                return fn

            # --- per-step ops, split per engine for precise stream order ---

            def mk_mm(w, t):
                # gate matmuls feeding step t (dh from step t-1); t>=1
                def fn():
                    st = state[w]
                    gates = st["gates"]
                    for g in range(4):
                        nc.tensor.matmul(
                            gates[:, ts(g, WAVE)], whht_s[:, ts(g, 128)],
                            st["dh"][:], start=False, stop=(t == T - 1),
                            skip_group_check=True,
                        )
                    if t == 1:
                        for g in range(4):
                            nc.tensor.matmul(
                                gates[:, ts(g, WAVE)], wes_s[:, ts(g, 128)],
                                st["k3rhs"][:],
                                start=False, stop=False, skip_group_check=True,
                            )
                return fn

            def mk_sig(w, t):
                def fn():
                    st = state[w]
                    sig = apool.tile([128, 4 * WAVE], f16, tag="sig", name="sig")
                    nc.scalar.activation(sig[:], st["gates"][:], AF.Sigmoid)
                    st["sig"] = sig
                return fn

            def mk_m1(w, t):
                def fn():
                    st = state[w]
                    sig = st["sig"]
                    m1 = apool.tile([128, 2 * WAVE], f16, tag="m1", name="m1")[:, :WAVE]
                    nc.vector.scalar_tensor_tensor(
                        m1[:], sig[:, 3 * WAVE : 4 * WAVE], 0.5, sig[:, 0:WAVE],
                        OP.subtract, OP.mult,
                    )
                    st["m1"] = m1
                return fn

            def mk_m2(w, t):
                def fn():
                    st = state[w]
                    m2 = apool.tile([128, 2 * WAVE], f16, tag="m2", name="m2")[:, :WAVE]
                    nc.gpsimd.tensor_tensor(
                        m2[:], st["sig"][:, WAVE : 2 * WAVE], st["ct_prev"][:],
                        OP.mult,
                    )
                    st["m2"] = m2
                return fn

            def mk_ct(w, t):
                def fn():
                    st = state[w]
                    ct = hcpool.tile([128, 2 * WAVE], f16, tag="ct", name="ct")[:, :WAVE]
                    if t == 0:
                        nc.vector.tensor_copy(ct[:], st["m1"][:])
                    else:
                        nc.vector.tensor_tensor(
                            ct[:], st["m1"][:], st["m2"][:], OP.add
                        )
                    st["ct"] = ct
                return fn

            def mk_tanh(w, t):
                def fn():
                    st = state[w]
                    tanhc = apool.tile([128, 2 * WAVE], f16, tag="tanhc", name="tanhc")[:, :WAVE]
                    nc.scalar.activation(tanhc[:], st["ct"][:], AF.Tanh, scale=2.0)
                    st["tanhc"] = tanhc
                return fn

            def mk_h(w, t):
                def fn():
                    st = state[w]
                    h_new = st["h_all"][:, ts(t, WAVE)]
                    nc.vector.tensor_tensor(
                        h_new[:], st["sig"][:, 2 * WAVE : 3 * WAVE], st["tanhc"][:],
                        OP.mult,
                    )
                    st["h_new"] = h_new
                return fn

            def mk_dh(w, t):
                def fn():
                    st = state[w]
                    dh = apool.tile([128, 2 * WAVE], f16, tag="dh", name="dh")[:, :WAVE]
                    nc.vector.tensor_tensor(
                        dh[:], st["h_new"][:], st["h_prev"][:], OP.subtract
                    )
                    st["dh"] = dh
                    st["h_prev"] = st["h_new"]
                    st["ct_prev"] = st["ct"]
                return fn

            def mk_rel_mm(w, q):
                def fn():
                    st = state[w]
                    psr2 = scrpool.tile([2, 2 * WAVE], f32, tag="scratch", name="psr2")
                    nc.tensor.matmul(
                        psr2[:], whpt_s[:], st["h_all"][:, ts(q, 2 * WAVE)],
                        start=True, stop=True,
                    )
                    dst = st["predsb"][:, ts(q, 2 * WAVE)]
                    if q % 2 == 0:
                        nc.vector.tensor_copy(dst, psr2[:])
                    else:
                        nc.scalar.copy(dst, psr2[:])
                return fn

            def mk_rel_out(w):
                def fn():
                    st = state[w]
                    nc.sync.dma_start(
                        pred_v[:, :, ts(w, WAVE)],
                        st["predsb"].rearrange("j (t b) -> j t b", b=WAVE),
                    )
                return fn

            # ---- schedule ----
            def mk_warm(i):
                # dummy matmuls while weights stream in: ramp the PE p-state
                # (0.65 -> 2.4 GHz needs ~3us of continuous execution)
                def fn():
                    wt = gatepool.tile([128, 4 * WAVE], f32, tag="gates", name="warm")
                    for r in range(4):
                        nc.tensor.matmul(
                            wt[:, ts(r, WAVE)], ind_s[:, 0:128],
                            ind_s[:, 0:WAVE], start=True, stop=True,
                            skip_group_check=True,
                        )
                return fn

            for i in range(4):
                events.append((i, 1, mk_warm(i)))

            Sv = [LEAD]
            for w in range(1, NW):
                Sv.append(Sv[-1] + STAG_PAT[(w - 1) % 3])

            # MLP fills right-aligned before each pair's chain start: they sit
            # between chain matmuls in the PE stream, pacing the three chains
            # ~1 micro apart (the in-order PE stream is the metronome).
            for p in range(NW // 2):
                pb = Sv[2 * p] - LEAD
                lo = max(0, pb - 12)
                hi = pb + 3
                span = hi - lo
                for j in range(8):
                    c1 = lo + span * (2 * j) // 15
                    c2 = lo + span * (2 * j + 1) // 15
                    events.append((c1, 50 + 2 * j, mk_mlp1_c(p, j, 0, 5)))
                    events.append((c2, 51 + 2 * j, mk_mlp1_c(p, j, 5, KT)))
                    events.append((c2 + 1, 36, mk_relu1(p, j)))
                events.append((pb + 5, 67, mk_mlp2_c(p, 0, 4)))
                events.append((pb + 6, 67, mk_mlp2_c(p, 4, 8)))
                events.append((pb + 7, 21, mk_relu2(p)))

            for w in range(NW):
                S = Sv[w]
                events.append((S - 2, 4, mk_init1(w)))
                events.append((S - 1, 4, mk_init2(w)))
                events.append((S - 1, 22, mk_k3rhs(w)))
                events.append((S, 2, mk_k3mm(w)))
                for t in range(T):
                    m = S + 3 * t
                    if t > 0:
                        events.append((m, 2, mk_mm(w, t)))
                    events.append((m, 10, mk_sig(w, t)))
                    events.append((m, 30, mk_m1(w, t)))
                    if t > 0:
                        events.append((m, 32, mk_m2(w, t)))
                    events.append((m + 1, 20, mk_ct(w, t)))
                    # tanh two sigma-slots after its own sigma: the in-order
                    # ACT stream then enforces a stable 1/3-period stagger of
                    # the three chains (sigA, tanhC', sigB, tanhA, sigC, ...)
                    events.append((m + 2, 9, mk_tanh(w, t)))
                    events.append((m + 2, 18, mk_h(w, t)))
                    if t < T - 1:
                        events.append((m + 2, 19, mk_dh(w, t)))
                for q in range(T // 2):
                    events.append((S + 3 * (T - 1) + 2 + q // 2, 80 + q % 2, mk_rel_mm(w, q)))
                events.append((S + 3 * (T - 1) + 5, 90, mk_rel_out(w)))

            for _, _, fn in sorted(events, key=lambda e: (e[0], e[1])):
                fn()

    nc.compile()
    return nc


def _prep(inputs):
    """Host-side weight folding + layout prep. Returns per-core input maps."""
    f = np.float64
    W_ih = np.asarray(inputs["W_ih"], f)
    W_hh = np.asarray(inputs["W_hh"], f)
    b_ih = np.asarray(inputs["b_ih"], f)
    b_hh = np.asarray(inputs["b_hh"], f)
    W1 = np.asarray(inputs["W1"], f)
    b1 = np.asarray(inputs["b1"], f)
    W2 = np.asarray(inputs["W2"], f)
    b2 = np.asarray(inputs["b2"], f)
    W_sp = np.asarray(inputs["W_sp"], f)
    b_sp = np.asarray(inputs["b_sp"], f)
    W_hp = np.asarray(inputs["W_hp"], f)
    b_hp = np.asarray(inputs["b_hp"], f)

    W_zx = W_ih[:, :ZX]
    W_emb = W_ih[:, ZX:]
    W_es = W_emb @ W_sp                       # [4H, 2]
    W_hh_f = W_hh + W_es @ W_hp               # [4H, H]
    bias1 = b_ih + b_hh + W_emb @ b_sp + W_es @ b_hp

    # reorder pytorch gates (i, f, g, o) -> (i, f, o, g)
    perm = np.r_[0:H, H : 2 * H, 3 * H : 4 * H, 2 * H : 3 * H]
    W_zx = W_zx[perm]
    W_hh_f = W_hh_f[perm]
    W_es = W_es[perm]
    bias1 = bias1[perm]
    # double the g-gate block: its bank then holds 2*g_pre, so
    # tanh(g) = 2*sigmoid(2*g_pre) - 1 comes out of the one big sigmoid
    dbl = np.ones((G4, 1))
    dbl[3 * H :] = 2.0
    W_zx = W_zx * dbl
    W_hh_f = W_hh_f * dbl
    W_es = W_es * dbl
    bias1 = bias1 * dbl[:, 0]

    def kxm(Wt, kp):  # [K, M] -> [128, K/128, M] fp16, zero-padded to kp rows
        K, M = Wt.shape
        out = np.zeros((kp, M), f)
        out[:K] = Wt
        return np.ascontiguousarray(
            out.reshape(kp // 128, 128, M).transpose(1, 0, 2)
        ).astype(np.float16)

    # b1 rides the constant-1 zx feature at row ZX; bias1 goes through the
    # bank-open matmul (start=True must cover whole PSUM banks)
    w1a = np.vstack([W1.T, b1[None, :]])      # [1057, 1024]

    consts = {
        "w1t": kxm(w1a, KP),
        "wzxt": kxm(W_zx.T, KP),
        "w2t": kxm(W2.T, MLP),
        "whht": np.ascontiguousarray(W_hh_f.T).astype(np.float16),
        "whpt": np.ascontiguousarray(W_hp.T).astype(np.float16),
        "k3": np.ascontiguousarray(-W_es.T).astype(np.float16),
        "wes": np.ascontiguousarray(W_es.T).astype(np.float16),
        # bank-open bias: bias2[r, bk*128+m] = bias1[(2*bk + r)*128 + m]
        "bias2": np.ascontiguousarray(
            bias1.reshape(4, 128).reshape(2, 2, 128).transpose(1, 0, 2).reshape(2, 256)
        ).astype(np.float16),
        # 0/1 indicator selecting which half-bank gets which bias row
        "ind": np.kron(np.eye(2), np.ones((1, WAVE))).astype(np.float16),
        "b2": b2.reshape(128, 1).astype(np.float32),
        "bhp": b_hp.reshape(2, 1).astype(np.float32),
    }

    enc = np.asarray(inputs["enc_h_feat"], np.float32)
    z = np.asarray(inputs["z"], np.float32)
    lpr = np.asarray(inputs["last_pos_rel"], np.float32)
    zxT = np.zeros((KP, B), np.float16)
    zxT[:MLP] = enc.T
    zxT[MLP:ZX] = z.T
    zxT[ZX] = 1.0
    lprT = np.ascontiguousarray(lpr.T).astype(np.float16)

    in_maps = []
    for c in range(NCORES):
        s = slice(c * BC, (c + 1) * BC)
        m = dict(consts)
        m["zxT"] = np.ascontiguousarray(zxT[:, s])
        m["lprT"] = np.ascontiguousarray(lprT[:, s])
        in_maps.append(m)
    return in_maps


def run(inputs, trace=False):
    from concourse.bass_utils import run_bass_kernel_spmd

    if "nc" not in _cache:
        _cache["nc"] = _build_nc()
    in_maps = _prep(inputs)
    res = run_bass_kernel_spmd(
        _cache["nc"], in_maps, core_ids=list(range(NCORES)), trace=trace
    )
    pred = np.concatenate([r["pred"] for r in res.results], axis=2)  # [T, 2, B]
    out = pred.transpose(0, 2, 1) + np.asarray(inputs["b_hp"], np.float32)[None, None, :]
    return np.ascontiguousarray(out), res


def kernel(**inputs) -> np.ndarray:
    out, _ = run(inputs, trace=False)
    return out


# revision 41
# speedup vs baseline: 1.0310x; 1.0310x over previous
"""Trainium2 Bass kernel for the trajectory-decoder LSTM problem.

Math (mirrors the reference, with algebraic folds):
  dec_inp feeds gates only through W_emb; W_sp/W_emb/W_hp collapse:
    W_es = W_emb @ W_sp            [4H, 2]
    gates_t = zx@W_zx.T + bias + r_{t-1}@W_es.T + h_{t-1}@W_hh.T
  For t>=1, r_{t-1} = h_{t-1}@W_hp.T + b_hp, so with
    W_hh' = W_hh + W_es @ W_hp,  bias1 = b_ih + b_hh + W_emb@b_sp + W_es@b_hp
  every step becomes uniform:  gates_t = zx@W_zx.T + bias1 + h_{t-1}@W_hh'.T
  plus a rank-2 step-0 correction (lpr - r_init)@W_es.T injected once.
  `last_pos` is dead code (output is just the stacked rel_pos).

  Cell-state folding: track ct = c/2.  With m1 = (sig(2g)-0.5)*sig(i)
  (= sig(i)*tanh(g)/2) and m2 = sig(f)*ct_prev:
    ct = m1 + m2,   tanh(c) = tanh(2*ct)  (ACT scale=2)
  so the c-update is a plain tensor_tensor add and no x2 op is needed.

  bias1 and b1 ride a constant-1 feature appended to zx (row ZX=1056 of
  the zxT layout): W_zx.T row 1056 = bias1, W1.T row 1056 = b1.  No
  bank-open bias matmuls, no bias adds on the vector engines.

Device strategy (pure data-parallel over 8 cores, 4096 batch each):
  - batch on the free dim, features on partitions
  - per 256-batch wave, the 4 gate pre-activations live RESIDENT in one
    2-bank PSUM tile; each step the PE accumulates (h_t - h_{t-1}) @ W_hh'.T
    into it (start=False).  First zx matmul per gate block opens the bank
    (start=True).
  - g-gate weights doubled on host so ONE sigmoid over all 4 blocks
    yields sig(i),sig(f),sig(o),sig(2g).
  - micro-round software pipeline: one chain-step per micro-round, three
    chains in flight (stagger 12 micros, step period 3 micros).  Engine
    assignment: ACT {sigmoid, tanh, half the rel copies}, DVE {m1, ct, h,
    mlp relu, other rel copies}, GPS {m2, dh}.  Emission order per engine
    is staggered so ops are ready when the engine reaches them.
  - all matmul operands fp16, PSUM accumulation fp32; b_hp added on host.
"""

import os
import numpy as np

B = 32768
NCORES = 8
BC = B // NCORES          # 4096 batch per core
WAVE = 256                # batch per recurrence chain (2 PSUM banks of gates)
NW = BC // WAVE           # 16 waves
PAIR = 2 * WAVE           # phase-A (mlp) runs at N=512 across wave pairs
T = 12                    # decode steps
H = 128
G4 = 4 * H                # 512 gate features
ZX = 1056
KP = 1152                 # ZX+1 (bias row) padded to 9*128
KT = KP // 128            # 9 contraction tiles
MLP = 1024
EMB = 64

# Stagger between chain starts: the step period is 3 micros, so the three
# in-flight chains must land on distinct micro residues mod 3, while
# S(w+3)-S(w) = 36 = chain span lets chain w+3 inherit chain w's rotation
# slot (and PSUM gates buffer) with no idle micros.  [13,13,10] does both.
STAG_PAT = [13, 13, 13]
LEAD = 11                 # micros of mlp warm-up before chain 0

_cache = {}


def _build_nc():
    import concourse.bass as bass
    import concourse.bacc as bacc
    import concourse.mybir as mybir
    import concourse.tile as tile
    from concourse.bass import ts

    f16 = mybir.dt.float16
    f32 = mybir.dt.float32
    AF = mybir.ActivationFunctionType
    OP = mybir.AluOpType

    nc = bacc.Bacc("TRN2", target_bir_lowering=False)

    zxT = nc.dram_tensor("zxT", [KP, BC], f16, kind="ExternalInput")
    lprT = nc.dram_tensor("lprT", [2, BC], f16, kind="ExternalInput")
    w1t = nc.dram_tensor("w1t", [128, KT, MLP], f16, kind="ExternalInput")
    wzxt = nc.dram_tensor("wzxt", [128, KT, G4], f16, kind="ExternalInput")
    w2t = nc.dram_tensor("w2t", [128, 8, H], f16, kind="ExternalInput")
    whht = nc.dram_tensor("whht", [128, G4], f16, kind="ExternalInput")
    whpt = nc.dram_tensor("whpt", [128, 2], f16, kind="ExternalInput")
    k3 = nc.dram_tensor("k3", [2, G4], f16, kind="ExternalInput")   # -W_es.T
    wes = nc.dram_tensor("wes", [2, G4], f16, kind="ExternalInput")  # +W_es.T
    bias2 = nc.dram_tensor("bias2", [2, 2 * 128], f16, kind="ExternalInput")
    ind = nc.dram_tensor("ind", [2, 2 * WAVE], f16, kind="ExternalInput")
    b2 = nc.dram_tensor("b2", [128, 1], f32, kind="ExternalInput")
    bhp = nc.dram_tensor("bhp", [2, 1], f32, kind="ExternalInput")
    pred = nc.dram_tensor("pred", [T, 2, BC], f32, kind="ExternalOutput")

    with tile.TileContext(nc) as tc:
        with (
            tc.tile_pool(name="consts", bufs=1) as cpool,
            tc.tile_pool(name="zx", bufs=2) as zxpool,
            tc.tile_pool(name="h1", bufs=2) as h1pool,
            tc.tile_pool(name="hc", bufs=10) as hcpool,
            tc.tile_pool(name="acts", bufs=6) as apool,
            tc.tile_pool(name="outs", bufs=4) as opool,
            tc.tile_pool(name="scrps", bufs=2, space="PSUM") as scrpool,
            tc.tile_pool(name="gateps", bufs=3, space="PSUM") as gatepool,
        ):
            # ---- load constants once (small ones first: the warmup matmuls
            # and first mlp chunks shouldn't wait behind megabyte weights) ----
            ind_s = cpool.tile([2, 2 * WAVE], f16)
            nc.sync.dma_start(ind_s[:], ind[:])
            bias2_s = cpool.tile([2, 2 * 128], f16)
            nc.sync.dma_start(bias2_s[:], bias2[:])
            whht_s = cpool.tile([128, G4], f16)
            nc.sync.dma_start(whht_s[:], whht[:])
            whpt_s = cpool.tile([128, 2], f16)
            nc.sync.dma_start(whpt_s[:], whpt[:])
            k3_s = cpool.tile([2, G4], f16)
            nc.sync.dma_start(k3_s[:], k3[:])
            wes_s = cpool.tile([2, G4], f16)
            nc.sync.dma_start(wes_s[:], wes[:])
            b2_s = cpool.tile([128, 1], f32)
            nc.sync.dma_start(b2_s[:], b2[:])
            bhp_s = cpool.tile([2, 1], f32)
            nc.sync.dma_start(bhp_s[:], bhp[:])
            lpr_s = cpool.tile([2, BC], f16)
            nc.sync.dma_start(lpr_s[:], lprT[:])
            w1t_s = cpool.tile([128, KT, MLP], f16)
            for j in range(8):
                nc.sync.dma_start(
                    w1t_s[:, :, ts(j, 128)], w1t[:, :, ts(j, 128)]
                )
            w2t_s = cpool.tile([128, 8, H], f16)
            nc.sync.dma_start(w2t_s[:], w2t[:])
            wzxt_s = cpool.tile([128, KT, G4], f16)
            nc.sync.dma_start(wzxt_s[:, :, 0 : 2 * 128], wzxt[:, :, 0 : 2 * 128])
            nc.sync.dma_start(
                wzxt_s[:, :, 2 * 128 : G4], wzxt[:, :, 2 * 128 : G4]
            )

            zxT_v = zxT.rearrange("(k p) b -> p k b", p=128)
            pred_v = pred.rearrange("t j b -> j t b")

            state = [dict() for _ in range(NW)]
            pair_state = [dict() for _ in range(NW // 2)]
            events = []  # (micro, prio, fn)

            def mk_mlp1_c(p, j, klo, khi):
                def fn():
                    st = pair_state[p]
                    if "zxw" not in st:
                        zxw = zxpool.tile([128, KT, PAIR], f16, tag="zxw", name="zxw")
                        nc.sync.dma_start(zxw[:], zxT_v[:, :, ts(p, PAIR)])
                        st["zxw"] = zxw
                        st["h1"] = h1pool.tile([128, 8, PAIR], f16, tag="h1", name="h1")
                    if klo == 0:
                        st["ps%d" % j] = scrpool.tile(
                            [128, PAIR], f32, tag="scratch", name="ps"
                        )
                    ps = st["ps%d" % j]
                    for k in range(klo, khi):
                        nc.tensor.matmul(
                            ps[:], w1t_s[:, k, ts(j, 128)], st["zxw"][:, k, :],
                            start=(k == 0), stop=(k == KT - 1),
                        )
                return fn

            def mk_relu1(p, j):
                def fn():
                    st = pair_state[p]
                    # b1 rides the constant-1 zx row: pure relu here
                    nc.vector.tensor_scalar_max(
                        st["h1"][:, j, :], st.pop("ps%d" % j)[:], 0.0
                    )
                return fn

            def mk_mlp2_c(p, jlo, jhi):
                def fn():
                    st = pair_state[p]
                    if jlo == 0:
                        st["ps2"] = scrpool.tile(
                            [128, PAIR], f32, tag="scratch", name="ps"
                        )
                    ps = st["ps2"]
                    for j in range(jlo, jhi):
                        nc.tensor.matmul(
                            ps[:], w2t_s[:, j, :], st["h1"][:, j, :],
                            start=(j == 0), stop=(j == 7),
                        )
                return fn

            def mk_relu2(p):
                def fn():
                    st = pair_state[p]
                    hi = h1pool.tile([128, PAIR], f16, tag="hinit", name="hinit")
                    nc.vector.tensor_scalar(
                        hi[:], st.pop("ps2")[:], b2_s[:, 0:1], 0.0, OP.add, OP.max
                    )
                    st["h_init"] = hi
                return fn

            def mk_init1(w):
                def fn():
                    st = state[w]
                    pst = pair_state[w // 2]
                    hs = ts(w % 2, WAVE)
                    st["h_prev"] = pst["h_init"][:, hs]
                    gates = gatepool.tile([128, 4 * WAVE], f32, tag="gates", name="gates")
                    st["gates"] = gates
                    # bank-open: start=True must cover a whole PSUM bank, so
                    # bias1 is injected via a K=2 matmul against a 0/1 mask
                    for bk in range(2):
                        nc.tensor.matmul(
                            gates[:, ts(bk, 2 * WAVE)], bias2_s[:, ts(bk, 128)],
                            ind_s[:], start=True, stop=False, skip_group_check=True,
                        )
                    for g in range(2):
                        gp = gates[:, ts(g, WAVE)]
                        for k in range(KT):
                            nc.tensor.matmul(
                                gp[:], wzxt_s[:, k, ts(g, 128)],
                                pst["zxw"][:, k, hs],
                                start=False, stop=False, skip_group_check=True,
                            )
                return fn

            def mk_init2(w):
                def fn():
                    st = state[w]
                    pst = pair_state[w // 2]
                    hs = ts(w % 2, WAVE)
                    gates = st["gates"]
                    for g in range(2, 4):
                        gp = gates[:, ts(g, WAVE)]
                        for k in range(KT):
                            nc.tensor.matmul(
                                gp[:], wzxt_s[:, k, ts(g, 128)],
                                pst["zxw"][:, k, hs],
                                start=False, stop=False, skip_group_check=True,
                            )
                    psr = scrpool.tile([2, WAVE], f32, tag="scratch", name="psr")
                    nc.tensor.matmul(
                        psr[:], whpt_s[:], st["h_prev"][:], start=True, stop=True
                    )
                    st["psr"] = psr
                    for g in range(4):
                        nc.tensor.matmul(
                            gates[:, ts(g, WAVE)], whht_s[:, ts(g, 128)],
                            st["h_prev"][:],
                            start=False, stop=False, skip_group_check=True,
                        )
                    st["predsb"] = opool.tile([2, T * WAVE], f32, tag="predsb", name="predsb")
                    st["h_all"] = opool.tile([128, T * WAVE], f16, tag="hall", name="hall")
                return fn

            def mk_k3rhs(w):
                def fn():
                    st = state[w]
                    k3rhs = apool.tile([2, WAVE], f16, tag="k3rhs", name="k3rhs")
                    nc.vector.scalar_tensor_tensor(
                        k3rhs[:], st.pop("psr")[:], bhp_s[:, 0:1],
                        lpr_s[:, ts(w, WAVE)], OP.add, OP.subtract,
                    )
                    st["k3rhs"] = k3rhs
                return fn

            def mk_k3mm(w):
                def fn():
                    st = state[w]
                    gates = st["gates"]
                    for g in range(4):
                        nc.tensor.matmul(
                            gates[:, ts(g, WAVE)], k3_s[:, ts(g, 128)],
                            st["k3rhs"][:],
                            start=False, stop=False, skip_group_check=True,
                        )
                return fn

            # --- per-step ops, split per engine for precise stream order ---

            def mk_mm(w, t):
                # gate matmuls feeding step t (dh from step t-1); t>=1
                def fn():
                    st = state[w]
                    gates = st["gates"]
                    for g in range(4):
                        nc.tensor.matmul(
                            gates[:, ts(g, WAVE)], whht_s[:, ts(g, 128)],
                            st["dh"][:], start=False, stop=(t == T - 1),
                            skip_group_check=True,
                        )
                    if t == 1:
                        for g in range(4):
                            nc.tensor.matmul(
                                gates[:, ts(g, WAVE)], wes_s[:, ts(g, 128)],
                                st["k3rhs"][:],
                                start=False, stop=False, skip_group_check=True,
                            )
                return fn

            def mk_sig(w, t):
                def fn():
                    st = state[w]
                    sig = apool.tile([128, 4 * WAVE], f16, tag="sig", name="sig")
                    nc.scalar.activation(sig[:], st["gates"][:], AF.Sigmoid)
                    st["sig"] = sig
                return fn

            def mk_m1(w, t):
                def fn():
                    st = state[w]
                    sig = st["sig"]
                    m1 = apool.tile([128, 2 * WAVE], f16, tag="m1", name="m1")[:, :WAVE]
                    nc.vector.scalar_tensor_tensor(
                        m1[:], sig[:, 3 * WAVE : 4 * WAVE], 0.5, sig[:, 0:WAVE],
                        OP.subtract, OP.mult,
                    )
                    st["m1"] = m1
                return fn

            def mk_m2(w, t):
                def fn():
                    st = state[w]
                    m2 = apool.tile([128, 2 * WAVE], f16, tag="m2", name="m2")[:, :WAVE]
                    nc.gpsimd.tensor_tensor(
                        m2[:], st["sig"][:, WAVE : 2 * WAVE], st["ct_prev"][:],
                        OP.mult,
                    )
                    st["m2"] = m2
                return fn

            def mk_ct(w, t):
                def fn():
                    st = state[w]
                    ct = hcpool.tile([128, 2 * WAVE], f16, tag="ct", name="ct")[:, :WAVE]
                    if t == 0:
                        nc.vector.tensor_copy(ct[:], st["m1"][:])
                    else:
                        nc.vector.tensor_tensor(
                            ct[:], st["m1"][:], st["m2"][:], OP.add
                        )
                    st["ct"] = ct
                return fn

            def mk_tanh(w, t):
                def fn():
                    st = state[w]
                    tanhc = apool.tile([128, 2 * WAVE], f16, tag="tanhc", name="tanhc")[:, :WAVE]
                    nc.scalar.activation(tanhc[:], st["ct"][:], AF.Tanh, scale=2.0)
                    st["tanhc"] = tanhc
                return fn

            def mk_h(w, t):
                def fn():
                    st = state[w]
                    h_new = st["h_all"][:, ts(t, WAVE)]
                    nc.vector.tensor_tensor(
                        h_new[:], st["sig"][:, 2 * WAVE : 3 * WAVE], st["tanhc"][:],
                        OP.mult,
                    )
                    st["h_new"] = h_new
                return fn

            def mk_dh(w, t):
                def fn():
                    st = state[w]
                    dh = apool.tile([128, 2 * WAVE], f16, tag="dh", name="dh")[:, :WAVE]
                    nc.vector.tensor_tensor(
                        dh[:], st["h_new"][:], st["h_prev"][:], OP.subtract
                    )
                    st["dh"] = dh
                    st["h_prev"] = st["h_new"]
                    st["ct_prev"] = st["ct"]
                return fn

            def mk_rel_mm(w, q):
                def fn():
                    st = state[w]
                    psr2 = scrpool.tile([2, 2 * WAVE], f32, tag="scratch", name="psr2")
                    nc.tensor.matmul(
                        psr2[:], whpt_s[:], st["h_all"][:, ts(q, 2 * WAVE)],
                        start=True, stop=True,
                    )
                    dst = st["predsb"][:, ts(q, 2 * WAVE)]
                    if q % 2 == 0:
                        nc.vector.tensor_copy(dst, psr2[:])
                    else:
                        nc.scalar.copy(dst, psr2[:])
                return fn

            def mk_rel_out(w):
                def fn():
                    st = state[w]
                    nc.sync.dma_start(
                        pred_v[:, :, ts(w, WAVE)],
                        st["predsb"].rearrange("j (t b) -> j t b", b=WAVE),
                    )
                return fn

            # ---- schedule ----
            def mk_warm(i):
                # dummy matmuls while weights stream in: ramp the PE p-state
                # (0.65 -> 2.4 GHz needs ~3us of continuous execution)
                def fn():
                    wt = gatepool.tile([128, 4 * WAVE], f32, tag="gates", name="warm")
                    for r in range(4):
                        nc.tensor.matmul(
                            wt[:, ts(r, WAVE)], ind_s[:, 0:128],
                            ind_s[:, 0:WAVE], start=True, stop=True,
                            skip_group_check=True,
                        )
                return fn

            for i in range(4):
                events.append((i, 1, mk_warm(i)))

            Sv = [LEAD]
            for w in range(1, NW):
                Sv.append(Sv[-1] + STAG_PAT[(w - 1) % 3])

            # MLP fills right-aligned before each pair's chain start: they sit
            # between chain matmuls in the PE stream, pacing the three chains
            # ~1 micro apart (the in-order PE stream is the metronome).
            for p in range(NW // 2):
                pb = Sv[2 * p] - LEAD
                lo = max(0, pb - 12)
                hi = pb + 3
                span = hi - lo
                for j in range(8):
                    c1 = lo + span * (2 * j) // 15
                    c2 = lo + span * (2 * j + 1) // 15
                    events.append((c1, 50 + 2 * j, mk_mlp1_c(p, j, 0, 5)))
                    events.append((c2, 51 + 2 * j, mk_mlp1_c(p, j, 5, KT)))
                    events.append((c2 + 1, 36, mk_relu1(p, j)))
                events.append((pb + 5, 67, mk_mlp2_c(p, 0, 4)))
                events.append((pb + 6, 67, mk_mlp2_c(p, 4, 8)))
                events.append((pb + 7, 21, mk_relu2(p)))

            for w in range(NW):
                S = Sv[w]
                events.append((S - 2, 4, mk_init1(w)))
                events.append((S - 1, 4, mk_init2(w)))
                events.append((S - 1, 22, mk_k3rhs(w)))
                events.append((S, 2, mk_k3mm(w)))
                for t in range(T):
                    m = S + 3 * t
                    if t > 0:
                        events.append((m, 2, mk_mm(w, t)))
                    events.append((m, 10, mk_sig(w, t)))
                    events.append((m, 30, mk_m1(w, t)))
                    if t > 0:
                        events.append((m, 32, mk_m2(w, t)))
                    events.append((m + 1, 20, mk_ct(w, t)))
                    # tanh two sigma-slots after its own sigma: the in-order
                    # ACT stream then enforces a stable 1/3-period stagger of
                    # the three chains (sigA, tanhC', sigB, tanhA, sigC, ...)
                    events.append((m + 2, 9, mk_tanh(w, t)))
                    events.append((m + 2, 18, mk_h(w, t)))
                    if t < T - 1:
                        events.append((m + 2, 19, mk_dh(w, t)))
                for q in range(T // 2):
                    events.append((S + 3 * (T - 1) + 2 + q // 2, 80 + q % 2, mk_rel_mm(w, q)))
                events.append((S + 3 * (T - 1) + 5, 90, mk_rel_out(w)))

            for _, _, fn in sorted(events, key=lambda e: (e[0], e[1])):
                fn()

    nc.compile()
    return nc


def _prep(inputs):
    """Host-side weight folding + layout prep. Returns per-core input maps."""
    f = np.float64
    W_ih = np.asarray(inputs["W_ih"], f)
    W_hh = np.asarray(inputs["W_hh"], f)
    b_ih = np.asarray(inputs["b_ih"], f)
    b_hh = np.asarray(inputs["b_hh"], f)
    W1 = np.asarray(inputs["W1"], f)
    b1 = np.asarray(inputs["b1"], f)
    W2 = np.asarray(inputs["W2"], f)
    b2 = np.asarray(inputs["b2"], f)
    W_sp = np.asarray(inputs["W_sp"], f)
    b_sp = np.asarray(inputs["b_sp"], f)
    W_hp = np.asarray(inputs["W_hp"], f)
    b_hp = np.asarray(inputs["b_hp"], f)

    W_zx = W_ih[:, :ZX]
    W_emb = W_ih[:, ZX:]
    W_es = W_emb @ W_sp                       # [4H, 2]
    W_hh_f = W_hh + W_es @ W_hp               # [4H, H]
    bias1 = b_ih + b_hh + W_emb @ b_sp + W_es @ b_hp

    # reorder pytorch gates (i, f, g, o) -> (i, f, o, g)
    perm = np.r_[0:H, H : 2 * H, 3 * H : 4 * H, 2 * H : 3 * H]
    W_zx = W_zx[perm]
    W_hh_f = W_hh_f[perm]
    W_es = W_es[perm]
    bias1 = bias1[perm]
    # double the g-gate block: its bank then holds 2*g_pre, so
    # tanh(g) = 2*sigmoid(2*g_pre) - 1 comes out of the one big sigmoid
    dbl = np.ones((G4, 1))
    dbl[3 * H :] = 2.0
    W_zx = W_zx * dbl
    W_hh_f = W_hh_f * dbl
    W_es = W_es * dbl
    bias1 = bias1 * dbl[:, 0]

    def kxm(Wt, kp):  # [K, M] -> [128, K/128, M] fp16, zero-padded to kp rows
        K, M = Wt.shape
        out = np.zeros((kp, M), f)
        out[:K] = Wt
        return np.ascontiguousarray(
            out.reshape(kp // 128, 128, M).transpose(1, 0, 2)
        ).astype(np.float16)

    # b1 rides the constant-1 zx feature at row ZX; bias1 goes through the
    # bank-open matmul (start=True must cover whole PSUM banks)
    w1a = np.vstack([W1.T, b1[None, :]])      # [1057, 1024]

    consts = {
        "w1t": kxm(w1a, KP),
        "wzxt": kxm(W_zx.T, KP),
        "w2t": kxm(W2.T, MLP),
        "whht": np.ascontiguousarray(W_hh_f.T).astype(np.float16),
        "whpt": np.ascontiguousarray(W_hp.T).astype(np.float16),
        "k3": np.ascontiguousarray(-W_es.T).astype(np.float16),
        "wes": np.ascontiguousarray(W_es.T).astype(np.float16),
        # bank-open bias: bias2[r, bk*128+m] = bias1[(2*bk + r)*128 + m]
        "bias2": np.ascontiguousarray(
            bias1.reshape(4, 128).reshape(2, 2, 128).transpose(1, 0, 2).reshape(2, 256)
        ).astype(np.float16),
        # 0/1 indicator selecting which half-bank gets which bias row
        "ind": np.kron(np.eye(2), np.ones((1, WAVE))).astype(np.float16),
        "b2": b2.reshape(128, 1).astype(np.float32),
        "bhp": b_hp.reshape(2, 1).astype(np.float32),
    }

    enc = np.asarray(inputs["enc_h_feat"], np.float32)
    z = np.asarray(inputs["z"], np.float32)
    lpr = np.asarray(inputs["last_pos_rel"], np.float32)
    zxT = np.zeros((KP, B), np.float16)
    zxT[:MLP] = enc.T
    zxT[MLP:ZX] = z.T
    zxT[ZX] = 1.0
    lprT = np.ascontiguousarray(lpr.T).astype(np.float16)

    in_maps = []
    for c in range(NCORES):
        s = slice(c * BC, (c + 1) * BC)
        m = dict(consts)
        m["zxT"] = np.ascontiguousarray(zxT[:, s])
        m["lprT"] = np.ascontiguousarray(lprT[:, s])
        in_maps.append(m)
    return in_maps


def run(inputs, trace=False):
    from concourse.bass_utils import run_bass_kernel_spmd

    if "nc" not in _cache:
        _cache["nc"] = _build_nc()
    in_maps = _prep(inputs)
    res = run_bass_kernel_spmd(
        _cache["nc"], in_maps, core_ids=list(range(NCORES)), trace=trace
    )
    pred = np.concatenate([r["pred"] for r in res.results], axis=2)  # [T, 2, B]
    out = pred.transpose(0, 2, 1) + np.asarray(inputs["b_hp"], np.float32)[None, None, :]
    return np.ascontiguousarray(out), res


def kernel(**inputs) -> np.ndarray:
    out, _ = run(inputs, trace=False)
    return out


# revision 45
# speedup vs baseline: 1.0584x; 1.0266x over previous
"""Trainium2 Bass kernel for the trajectory-decoder LSTM problem.

Math (mirrors the reference, with algebraic folds):
  dec_inp feeds gates only through W_emb; W_sp/W_emb/W_hp collapse:
    W_es = W_emb @ W_sp            [4H, 2]
    gates_t = zx@W_zx.T + bias + r_{t-1}@W_es.T + h_{t-1}@W_hh.T
  For t>=1, r_{t-1} = h_{t-1}@W_hp.T + b_hp, so with
    W_hh' = W_hh + W_es @ W_hp,  bias1 = b_ih + b_hh + W_emb@b_sp + W_es@b_hp
  every step becomes uniform:  gates_t = zx@W_zx.T + bias1 + h_{t-1}@W_hh'.T
  plus a rank-2 step-0 correction (lpr - r_init)@W_es.T injected once.
  `last_pos` is dead code (output is just the stacked rel_pos).

  Cell-state folding: track ct = c/2.  With m1 = (sig(2g)-0.5)*sig(i)
  (= sig(i)*tanh(g)/2) and m2 = sig(f)*ct_prev:
    ct = m1 + m2,   tanh(c) = tanh(2*ct)  (ACT scale=2)
  so the c-update is a plain tensor_tensor add and no x2 op is needed.

  bias1 and b1 ride a constant-1 feature appended to zx (row ZX=1056 of
  the zxT layout): W_zx.T row 1056 = bias1, W1.T row 1056 = b1.  No
  bank-open bias matmuls, no bias adds on the vector engines.

Device strategy (pure data-parallel over 8 cores, 4096 batch each):
  - batch on the free dim, features on partitions
  - per 256-batch wave, the 4 gate pre-activations live RESIDENT in one
    2-bank PSUM tile; each step the PE accumulates (h_t - h_{t-1}) @ W_hh'.T
    into it (start=False).  First zx matmul per gate block opens the bank
    (start=True).
  - g-gate weights doubled on host so ONE sigmoid over all 4 blocks
    yields sig(i),sig(f),sig(o),sig(2g).
  - micro-round software pipeline: one chain-step per micro-round, three
    chains in flight (stagger 12 micros, step period 3 micros).  Engine
    assignment: ACT {sigmoid, tanh, half the rel copies}, DVE {m1, ct, h,
    mlp relu, other rel copies}, GPS {m2, dh}.  Emission order per engine
    is staggered so ops are ready when the engine reaches them.
  - all matmul operands fp16, PSUM accumulation fp32; b_hp added on host.
"""

import os
import numpy as np

B = 32768
NCORES = 8
BC = B // NCORES          # 4096 batch per core
WAVE = 256                # batch per recurrence chain (2 PSUM banks of gates)
NW = BC // WAVE           # 16 waves
PAIR = 2 * WAVE           # phase-A (mlp) runs at N=512 across wave pairs
T = 12                    # decode steps
H = 128
G4 = 4 * H                # 512 gate features
ZX = 1056
KP = 1152                 # ZX+1 (bias row) padded to 9*128
KT = KP // 128            # 9 contraction tiles
MLP = 1024
EMB = 64

# Stagger between chain starts: the step period is 3 micros, so the three
# in-flight chains must land on distinct micro residues mod 3, while
# S(w+3)-S(w) = 36 = chain span lets chain w+3 inherit chain w's rotation
# slot (and PSUM gates buffer) with no idle micros.  [13,13,10] does both.
STAG_PAT = [13, 13, 13]
LEAD = 11                 # micros of mlp warm-up before chain 0

_cache = {}


def _build_nc():
    import concourse.bass as bass
    import concourse.bacc as bacc
    import concourse.mybir as mybir
    import concourse.tile as tile
    from concourse.bass import ts

    f16 = mybir.dt.float16
    f32 = mybir.dt.float32
    AF = mybir.ActivationFunctionType
    OP = mybir.AluOpType

    nc = bacc.Bacc("TRN2", target_bir_lowering=False)

    zxT = nc.dram_tensor("zxT", [KP, BC], f16, kind="ExternalInput")
    w1t = nc.dram_tensor("w1t", [128, KT, MLP], f16, kind="ExternalInput")
    wzxt = nc.dram_tensor("wzxt", [128, KT, G4], f16, kind="ExternalInput")
    w2t = nc.dram_tensor("w2t", [128, 8, H], f16, kind="ExternalInput")
    # small per-partition consts packed into two tensors: one DMA each
    # pk128: [whht f16 1024B | whpt f16 4B | b2 f32 4B] = 1032B/partition
    # pk2:   [ind 1024 | bias2 512 | k3 1024 | wes 1024 | bhp 8 | lpr 8192]
    pk128 = nc.dram_tensor("pk128", [128, 1032], mybir.dt.uint8, kind="ExternalInput")
    pk2 = nc.dram_tensor("pk2", [2, 11784], mybir.dt.uint8, kind="ExternalInput")
    pred = nc.dram_tensor("pred", [T, 2, BC], f32, kind="ExternalOutput")

    with tile.TileContext(nc) as tc:
        with (
            tc.tile_pool(name="consts", bufs=1) as cpool,
            tc.tile_pool(name="zx", bufs=2) as zxpool,
            tc.tile_pool(name="h1", bufs=2) as h1pool,
            tc.tile_pool(name="hc", bufs=10) as hcpool,
            tc.tile_pool(name="acts", bufs=6) as apool,
            tc.tile_pool(name="outs", bufs=4) as opool,
            tc.tile_pool(name="scrps", bufs=2, space="PSUM") as scrpool,
            tc.tile_pool(name="gateps", bufs=3, space="PSUM") as gatepool,
        ):
            # ---- load constants: two packed DMAs for the small stuff, so
            # sync-engine dispatch (0.6-1.4us each!) doesn't serialize the
            # warm-up, then the big weights ----
            pk2_s = cpool.tile([2, 11784], mybir.dt.uint8)
            nc.sync.dma_start(pk2_s[:], pk2[:])
            pk128_s = cpool.tile([128, 1032], mybir.dt.uint8)
            nc.sync.dma_start(pk128_s[:], pk128[:])
            ind_s = pk2_s[:, 0:1024].bitcast(f16)
            bias2_s = pk2_s[:, 1024:1536].bitcast(f16)
            k3_s = pk2_s[:, 1536:2560].bitcast(f16)
            wes_s = pk2_s[:, 2560:3584].bitcast(f16)
            bhp_s = pk2_s[:, 3584:3592].bitcast(f32)
            lpr_s = pk2_s[:, 3592:11784].bitcast(f16)
            whht_s = pk128_s[:, 0:1024].bitcast(f16)
            whpt_s = pk128_s[:, 1024:1028].bitcast(f16)
            b2_s = pk128_s[:, 1028:1032].bitcast(f32)
            w1t_s = cpool.tile([128, KT, MLP], f16)
            nc.sync.dma_start(w1t_s[:], w1t[:])
            wzxt_s = cpool.tile([128, KT, G4], f16)
            nc.sync.dma_start(wzxt_s[:], wzxt[:])
            w2t_s = cpool.tile([128, 8, H], f16)
            nc.sync.dma_start(w2t_s[:], w2t[:])

            zxT_v = zxT.rearrange("(k p) b -> p k b", p=128)
            pred_v = pred.rearrange("t j b -> j t b")

            state = [dict() for _ in range(NW)]
            pair_state = [dict() for _ in range(NW // 2)]
            events = []  # (micro, prio, fn)

            def mk_mlp1_c(p, j, klo, khi):
                def fn():
                    st = pair_state[p]
                    if "zxw" not in st:
                        zxw = zxpool.tile([128, KT, PAIR], f16, tag="zxw", name="zxw")
                        nc.sync.dma_start(zxw[:], zxT_v[:, :, ts(p, PAIR)])
                        st["zxw"] = zxw
                        st["h1"] = h1pool.tile([128, 8, PAIR], f16, tag="h1", name="h1")
                    if klo == 0:
                        st["ps%d" % j] = scrpool.tile(
                            [128, PAIR], f32, tag="scratch", name="ps"
                        )
                    ps = st["ps%d" % j]
                    for k in range(klo, khi):
                        nc.tensor.matmul(
                            ps[:], w1t_s[:, k, ts(j, 128)], st["zxw"][:, k, :],
                            start=(k == 0), stop=(k == KT - 1),
                        )
                return fn

            def mk_relu1(p, j):
                def fn():
                    st = pair_state[p]
                    # b1 rides the constant-1 zx row: pure relu here
                    nc.vector.tensor_scalar_max(
                        st["h1"][:, j, :], st.pop("ps%d" % j)[:], 0.0
                    )
                return fn

            def mk_mlp2_c(p, jlo, jhi):
                def fn():
                    st = pair_state[p]
                    if jlo == 0:
                        st["ps2"] = scrpool.tile(
                            [128, PAIR], f32, tag="scratch", name="ps"
                        )
                    ps = st["ps2"]
                    for j in range(jlo, jhi):
                        nc.tensor.matmul(
                            ps[:], w2t_s[:, j, :], st["h1"][:, j, :],
                            start=(j == 0), stop=(j == 7),
                        )
                return fn

            def mk_relu2(p):
                def fn():
                    st = pair_state[p]
                    hi = h1pool.tile([128, PAIR], f16, tag="hinit", name="hinit")
                    nc.vector.tensor_scalar(
                        hi[:], st.pop("ps2")[:], b2_s[:, 0:1], 0.0, OP.add, OP.max
                    )
                    st["h_init"] = hi
                return fn

            def mk_init_open(w):
                def fn():
                    st = state[w]
                    gates = gatepool.tile([128, 4 * WAVE], f32, tag="gates", name="gates")
                    st["gates"] = gates
                    # bank-open: start=True must cover a whole PSUM bank, so
                    # bias1 is injected via a K=2 matmul against a 0/1 mask
                    for bk in range(2):
                        nc.tensor.matmul(
                            gates[:, ts(bk, 2 * WAVE)], bias2_s[:, ts(bk, 128)],
                            ind_s[:], start=True, stop=False, skip_group_check=True,
                        )
                return fn

            def mk_init_zx(w, g):
                def fn():
                    st = state[w]
                    pst = pair_state[w // 2]
                    hs = ts(w % 2, WAVE)
                    gp = st["gates"][:, ts(g, WAVE)]
                    for k in range(KT):
                        nc.tensor.matmul(
                            gp[:], wzxt_s[:, k, ts(g, 128)],
                            pst["zxw"][:, k, hs],
                            start=False, stop=False, skip_group_check=True,
                        )
                return fn

            def mk_init_h(w):
                def fn():
                    st = state[w]
                    pst = pair_state[w // 2]
                    hs = ts(w % 2, WAVE)
                    st["h_prev"] = pst["h_init"][:, hs]
                    gates = st["gates"]
                    psr = scrpool.tile([2, WAVE], f32, tag="scratch", name="psr")
                    nc.tensor.matmul(
                        psr[:], whpt_s[:], st["h_prev"][:], start=True, stop=True
                    )
                    st["psr"] = psr
                    for g in range(4):
                        nc.tensor.matmul(
                            gates[:, ts(g, WAVE)], whht_s[:, ts(g, 128)],
                            st["h_prev"][:],
                            start=False, stop=False, skip_group_check=True,
                        )
                    st["predsb"] = opool.tile([2, T * WAVE], f32, tag="predsb", name="predsb")
                    st["h_all"] = opool.tile([128, T * WAVE], f16, tag="hall", name="hall")
                return fn

            def mk_k3rhs(w):
                def fn():
                    st = state[w]
                    k3rhs = apool.tile([2, WAVE], f16, tag="k3rhs", name="k3rhs")
                    nc.vector.scalar_tensor_tensor(
                        k3rhs[:], st.pop("psr")[:], bhp_s[:, 0:1],
                        lpr_s[:, ts(w, WAVE)], OP.add, OP.subtract,
                    )
                    st["k3rhs"] = k3rhs
                return fn

            def mk_k3mm(w):
                def fn():
                    st = state[w]
                    gates = st["gates"]
                    for g in range(4):
                        nc.tensor.matmul(
                            gates[:, ts(g, WAVE)], k3_s[:, ts(g, 128)],
                            st["k3rhs"][:],
                            start=False, stop=False, skip_group_check=True,
                        )
                return fn

            # --- per-step ops, split per engine for precise stream order ---

            def mk_mm(w, t):
                # gate matmuls feeding step t (dh from step t-1); t>=1
                def fn():
                    st = state[w]
                    gates = st["gates"]
                    for g in range(4):
                        nc.tensor.matmul(
                            gates[:, ts(g, WAVE)], whht_s[:, ts(g, 128)],
                            st["dh"][:], start=False, stop=(t == T - 1),
                            skip_group_check=True,
                        )
                    if t == 1:
                        for g in range(4):
                            nc.tensor.matmul(
                                gates[:, ts(g, WAVE)], wes_s[:, ts(g, 128)],
                                st["k3rhs"][:],
                                start=False, stop=False, skip_group_check=True,
                            )
                return fn

            def mk_sig(w, t):
                def fn():
                    st = state[w]
                    sig = apool.tile([128, 4 * WAVE], f16, tag="sig", name="sig")
                    nc.scalar.activation(sig[:], st["gates"][:], AF.Sigmoid)
                    st["sig"] = sig
                return fn

            def mk_m1(w, t):
                def fn():
                    st = state[w]
                    sig = st["sig"]
                    m1 = apool.tile([128, 2 * WAVE], f16, tag="m1", name="m1")[:, :WAVE]
                    nc.vector.scalar_tensor_tensor(
                        m1[:], sig[:, 3 * WAVE : 4 * WAVE], 0.5, sig[:, 0:WAVE],
                        OP.subtract, OP.mult,
                    )
                    st["m1"] = m1
                return fn

            def mk_m2(w, t):
                def fn():
                    st = state[w]
                    m2 = apool.tile([128, 2 * WAVE], f16, tag="m2", name="m2")[:, :WAVE]
                    nc.gpsimd.tensor_tensor(
                        m2[:], st["sig"][:, WAVE : 2 * WAVE], st["ct_prev"][:],
                        OP.mult,
                    )
                    st["m2"] = m2
                return fn

            def mk_ct(w, t):
                def fn():
                    st = state[w]
                    ct = hcpool.tile([128, 2 * WAVE], f16, tag="ct", name="ct")[:, :WAVE]
                    if t == 0:
                        nc.vector.tensor_copy(ct[:], st["m1"][:])
                    else:
                        nc.vector.tensor_tensor(
                            ct[:], st["m1"][:], st["m2"][:], OP.add
                        )
                    st["ct"] = ct
                return fn

            def mk_tanh(w, t):
                def fn():
                    st = state[w]
                    tanhc = apool.tile([128, 2 * WAVE], f16, tag="tanhc", name="tanhc")[:, :WAVE]
                    nc.scalar.activation(tanhc[:], st["ct"][:], AF.Tanh, scale=2.0)
                    st["tanhc"] = tanhc
                return fn

            def mk_h(w, t):
                def fn():
                    st = state[w]
                    h_new = st["h_all"][:, ts(t, WAVE)]
                    nc.vector.tensor_tensor(
                        h_new[:], st["sig"][:, 2 * WAVE : 3 * WAVE], st["tanhc"][:],
                        OP.mult,
                    )
                    st["h_new"] = h_new
                return fn

            def mk_dh(w, t):
                def fn():
                    st = state[w]
                    dh = apool.tile([128, 2 * WAVE], f16, tag="dh", name="dh")[:, :WAVE]
                    nc.vector.tensor_tensor(
                        dh[:], st["h_new"][:], st["h_prev"][:], OP.subtract
                    )
                    st["dh"] = dh
                    st["h_prev"] = st["h_new"]
                    st["ct_prev"] = st["ct"]
                return fn

            def mk_rel_mm(w, q):
                def fn():
                    st = state[w]
                    psr2 = scrpool.tile([2, 2 * WAVE], f32, tag="scratch", name="psr2")
                    nc.tensor.matmul(
                        psr2[:], whpt_s[:], st["h_all"][:, ts(q, 2 * WAVE)],
                        start=True, stop=True,
                    )
                    dst = st["predsb"][:, ts(q, 2 * WAVE)]
                    if q % 2 == 0:
                        nc.vector.tensor_copy(dst, psr2[:])
                    else:
                        nc.scalar.copy(dst, psr2[:])
                return fn

            def mk_rel_out(w):
                def fn():
                    st = state[w]
                    nc.sync.dma_start(
                        pred_v[:, :, ts(w, WAVE)],
                        st["predsb"].rearrange("j (t b) -> j t b", b=WAVE),
                    )
                return fn

            # ---- schedule ----
            def mk_warm(i):
                # dummy matmuls while weights stream in: ramp the PE p-state
                # (0.65 -> 2.4 GHz needs ~3us of continuous execution)
                def fn():
                    wt = gatepool.tile([128, 4 * WAVE], f32, tag="gates", name="warm")
                    for r in range(4):
                        nc.tensor.matmul(
                            wt[:, ts(r, WAVE)], ind_s[:, 0:128],
                            ind_s[:, 0:WAVE], start=True, stop=True,
                            skip_group_check=True,
                        )
                return fn

            def mk_zxw_prefetch(p):
                def fn():
                    st = pair_state[p]
                    zxw = zxpool.tile([128, KT, PAIR], f16, tag="zxw", name="zxw")
                    nc.sync.dma_start(zxw[:], zxT_v[:, :, ts(p, PAIR)])
                    st["zxw"] = zxw
                    st["h1"] = h1pool.tile([128, 8, PAIR], f16, tag="h1", name="h1")
                return fn

            events.append((0, 0, mk_zxw_prefetch(0)))
            for i in range(4):
                events.append((i, 1, mk_warm(i)))

            Sv = [LEAD]
            for w in range(1, NW):
                Sv.append(Sv[-1] + STAG_PAT[(w - 1) % 3])

            # MLP fills right-aligned before each pair's chain start: they sit
            # between chain matmuls in the PE stream, pacing the three chains
            # ~1 micro apart (the in-order PE stream is the metronome).
            for p in range(NW // 2):
                pb = Sv[2 * p] - LEAD
                lo = max(0, pb - 12)
                hi = pb + 3
                span = hi - lo
                for j in range(8):
                    c1 = lo + span * (2 * j) // 15
                    c2 = lo + span * (2 * j + 1) // 15
                    events.append((c1, 50 + 2 * j, mk_mlp1_c(p, j, 0, 5)))
                    events.append((c2, 51 + 2 * j, mk_mlp1_c(p, j, 5, KT)))
                    events.append((c2 + 1, 36, mk_relu1(p, j)))
                events.append((pb + 5, 67, mk_mlp2_c(p, 0, 4)))
                events.append((pb + 6, 67, mk_mlp2_c(p, 4, 8)))
                events.append((pb + 7, 21, mk_relu2(p)))

            for w in range(NW):
                S = Sv[w]
                events.append((S - 4, 4, mk_init_open(w)))
                events.append((S - 4, 5, mk_init_zx(w, 0)))
                events.append((S - 3, 4, mk_init_zx(w, 1)))
                events.append((S - 2, 4, mk_init_zx(w, 2)))
                events.append((S - 1, 4, mk_init_zx(w, 3)))
                events.append((S - 1, 5, mk_init_h(w)))
                events.append((S - 1, 22, mk_k3rhs(w)))
                events.append((S, 2, mk_k3mm(w)))
                for t in range(T):
                    m = S + 3 * t
                    if t > 0:
                        events.append((m, 2, mk_mm(w, t)))
                    events.append((m, 10, mk_sig(w, t)))
                    events.append((m, 30, mk_m1(w, t)))
                    if t > 0:
                        events.append((m, 32, mk_m2(w, t)))
                    events.append((m + 1, 20, mk_ct(w, t)))
                    # tanh two sigma-slots after its own sigma: the in-order
                    # ACT stream then enforces a stable 1/3-period stagger of
                    # the three chains (sigA, tanhC', sigB, tanhA, sigC, ...)
                    events.append((m + 2, 9, mk_tanh(w, t)))
                    events.append((m + 2, 18, mk_h(w, t)))
                    if t < T - 1:
                        events.append((m + 2, 19, mk_dh(w, t)))
                for q in range(T // 2):
                    events.append((S + 3 * (T - 1) + 2 + q // 2, 80 + q % 2, mk_rel_mm(w, q)))
                events.append((S + 3 * (T - 1) + 5, 90, mk_rel_out(w)))

            for _, _, fn in sorted(events, key=lambda e: (e[0], e[1])):
                fn()

    nc.compile()
    return nc


def _prep(inputs):
    """Host-side weight folding + layout prep. Returns per-core input maps."""
    f = np.float64
    W_ih = np.asarray(inputs["W_ih"], f)
    W_hh = np.asarray(inputs["W_hh"], f)
    b_ih = np.asarray(inputs["b_ih"], f)
    b_hh = np.asarray(inputs["b_hh"], f)
    W1 = np.asarray(inputs["W1"], f)
    b1 = np.asarray(inputs["b1"], f)
    W2 = np.asarray(inputs["W2"], f)
    b2 = np.asarray(inputs["b2"], f)
    W_sp = np.asarray(inputs["W_sp"], f)
    b_sp = np.asarray(inputs["b_sp"], f)
    W_hp = np.asarray(inputs["W_hp"], f)
    b_hp = np.asarray(inputs["b_hp"], f)

    W_zx = W_ih[:, :ZX]
    W_emb = W_ih[:, ZX:]
    W_es = W_emb @ W_sp                       # [4H, 2]
    W_hh_f = W_hh + W_es @ W_hp               # [4H, H]
    bias1 = b_ih + b_hh + W_emb @ b_sp + W_es @ b_hp

    # reorder pytorch gates (i, f, g, o) -> (i, f, o, g)
    perm = np.r_[0:H, H : 2 * H, 3 * H : 4 * H, 2 * H : 3 * H]
    W_zx = W_zx[perm]
    W_hh_f = W_hh_f[perm]
    W_es = W_es[perm]
    bias1 = bias1[perm]
    # double the g-gate block: its bank then holds 2*g_pre, so
    # tanh(g) = 2*sigmoid(2*g_pre) - 1 comes out of the one big sigmoid
    dbl = np.ones((G4, 1))
    dbl[3 * H :] = 2.0
    W_zx = W_zx * dbl
    W_hh_f = W_hh_f * dbl
    W_es = W_es * dbl
    bias1 = bias1 * dbl[:, 0]

    def kxm(Wt, kp):  # [K, M] -> [128, K/128, M] fp16, zero-padded to kp rows
        K, M = Wt.shape
        out = np.zeros((kp, M), f)
        out[:K] = Wt
        return np.ascontiguousarray(
            out.reshape(kp // 128, 128, M).transpose(1, 0, 2)
        ).astype(np.float16)

    # b1 rides the constant-1 zx feature at row ZX; bias1 goes through the
    # bank-open matmul (start=True must cover whole PSUM banks)
    w1a = np.vstack([W1.T, b1[None, :]])      # [1057, 1024]

    # pk128: [whht f16 1024B | whpt f16 4B | b2 f32 4B]
    pk128 = np.zeros((128, 1032), np.uint8)
    pk128[:, 0:1024] = np.ascontiguousarray(W_hh_f.T).astype(np.float16).view(np.uint8)
    pk128[:, 1024:1028] = np.ascontiguousarray(W_hp.T).astype(np.float16).view(np.uint8)
    pk128[:, 1028:1032] = b2.reshape(128, 1).astype(np.float32).view(np.uint8)

    # bank-open bias: bias2[r, bk*128+m] = bias1[(2*bk + r)*128 + m]
    bias2 = np.ascontiguousarray(
        bias1.reshape(4, 128).reshape(2, 2, 128).transpose(1, 0, 2).reshape(2, 256)
    ).astype(np.float16)
    ind = np.kron(np.eye(2), np.ones((1, WAVE))).astype(np.float16)

    consts = {
        "w1t": kxm(w1a, KP),
        "wzxt": kxm(W_zx.T, KP),
        "w2t": kxm(W2.T, MLP),
        "pk128": pk128,
    }

    enc = np.asarray(inputs["enc_h_feat"], np.float32)
    z = np.asarray(inputs["z"], np.float32)
    lpr = np.asarray(inputs["last_pos_rel"], np.float32)
    zxT = np.zeros((KP, B), np.float16)
    zxT[:MLP] = enc.T
    zxT[MLP:ZX] = z.T
    zxT[ZX] = 1.0
    lprT = np.ascontiguousarray(lpr.T).astype(np.float16)

    in_maps = []
    for c in range(NCORES):
        s = slice(c * BC, (c + 1) * BC)
        m = dict(consts)
        m["zxT"] = np.ascontiguousarray(zxT[:, s])
        # pk2: [ind 1024 | bias2 512 | k3 1024 | wes 1024 | bhp 8 | lpr 8192]
        pk2 = np.zeros((2, 11784), np.uint8)
        pk2[:, 0:1024] = ind.view(np.uint8)
        pk2[:, 1024:1536] = bias2.view(np.uint8)
        pk2[:, 1536:2560] = np.ascontiguousarray(-W_es.T).astype(np.float16).view(np.uint8)
        pk2[:, 2560:3584] = np.ascontiguousarray(W_es.T).astype(np.float16).view(np.uint8)
        pk2[:, 3584:3588] = b_hp.reshape(2, 1).astype(np.float32).view(np.uint8)
        pk2[:, 3592:11784] = np.ascontiguousarray(lprT[:, s]).view(np.uint8)
        m["pk2"] = pk2
        in_maps.append(m)
    return in_maps


def run(inputs, trace=False):
    from concourse.bass_utils import run_bass_kernel_spmd

    if "nc" not in _cache:
        _cache["nc"] = _build_nc()
    in_maps = _prep(inputs)
    res = run_bass_kernel_spmd(
        _cache["nc"], in_maps, core_ids=list(range(NCORES)), trace=trace
    )
    pred = np.concatenate([r["pred"] for r in res.results], axis=2)  # [T, 2, B]
    out = pred.transpose(0, 2, 1) + np.asarray(inputs["b_hp"], np.float32)[None, None, :]
    return np.ascontiguousarray(out), res


def kernel(**inputs) -> np.ndarray:
    out, _ = run(inputs, trace=False)
    return out
